# revision 14
# baseline (speedup 1.0000x reference)
"""Trainium2 Bass kernel for nn_MultiHeadAttention_8040178778165.

Causal multi-head attention (B=4, T=2048, C=1024, H=16) with RoPE,
tensor-parallel over heads: each of the 8 NeuronCores owns 2 heads.

v3: software-pipelined emission (attn(b) overlaps proj(b+1) and
out(b-1)); q/k projection via 3-pass split-fp8 DoubleRow matmuls
(x = x_hi + x_lo, 32W = W_hi + W_lo, dropping the lo*lo term) which is
bf16-accurate at half the PE cost; RoPE pair-swap via a permutation
matmul on the PE; osc -> ao transpose on the DMA XBAR engines; batched
DMAs. The attention core (S, exp, O accumulation) runs in bf16.

Per-core pipeline:
  - q/k projection (fp8 split DoubleRow), v projection token-major in
    bf16 straight into vb (x stationary).
  - RoPE: xbar = qkv/32 + bias (DVE, PSUM->SBUF), u/t1 = xbar*sin/cos
    (Pool, SBUF only), usw = Pi @ u (PE, deferred one step), dest =
    t1 + usw (DVE).
  - Flash-style causal attention per (batch, head): S^T tiles on PE,
    exp on ScalarE with a -2 bias (denominator-invariant), O
    accumulated per kt-pair; ones-column in vb produces softmax
    denominators.
  - Per-partition reciprocal scale into bf16 osc, DMA-XBAR transpose to
    channel-major ao, output projection against this core's 128 rows
    of Wout (bf16).
Host sums the 8 partial y^T outputs and adds biases.
"""

import sys

sys.path.insert(0, "/opt/trn_rl_repo")

import numpy as np
import ml_dtypes

import concourse.bacc as bacc
import concourse.mybir as mybir
import concourse.tile as tile
from concourse.bass_utils import run_bass_kernel_spmd

F32 = mybir.dt.float32
F32R = mybir.dt.float32r
BF16 = mybir.dt.bfloat16
FP8 = mybir.dt.float8e4
AX = mybir.AluOpType
DR = mybir.MatmulPerfMode.DoubleRow

B, T, C, H = 4, 2048, 1024, 16
HS = C // H            # 64
NT = B * T             # 8192
NCORES = 8
HPC = H // NCORES      # heads per core = 2
EXP_SHIFT = -2.0       # uniform exp shift; cancels in softmax normalization
WSCALE = 32.0          # Wqkv prescale so fp8 split stays in normal range

SPLIT_QK = True        # q/k proj via 3-pass split-fp8 DoubleRow


def build_nc(debug=False):
    nc = bacc.Bacc()

    if SPLIT_QK:
        xh = nc.declare_dram_parameter("xh", [128, 8, NT], FP8, isOutput=False)
        xl = nc.declare_dram_parameter("xl", [128, 8, NT], FP8, isOutput=False)
        wqkh = nc.declare_dram_parameter("wqkh", [128, 8, 256], FP8, isOutput=False)
        wqkl = nc.declare_dram_parameter("wqkl", [128, 8, 256], FP8, isOutput=False)
    else:
        wqkb = nc.declare_dram_parameter("wqkb", [128, 8, 256], BF16, isOutput=False)
    xb = nc.declare_dram_parameter("xb", [128, 8, NT], BF16, isOutput=False)
    wv = nc.declare_dram_parameter("wv", [128, 8, 128], BF16, isOutput=False)
    wo = nc.declare_dram_parameter("wo", [128, C], BF16, isOutput=False)
    bqk = nc.declare_dram_parameter("bqk", [128, 2], F32, isOutput=False)
    cosT = nc.declare_dram_parameter("cosT", [128, T], BF16, isOutput=False)
    sinP = nc.declare_dram_parameter("sinP", [128, T], BF16, isOutput=False)
    pim = nc.declare_dram_parameter("pim", [128, 128], BF16, isOutput=False)
    yT = nc.declare_dram_parameter("yT", [C, NT], BF16, isOutput=True)
    if debug:
        dbg_qT = nc.declare_dram_parameter("dbg_qT", [128, T], BF16, isOutput=True)
        dbg_kT = nc.declare_dram_parameter("dbg_kT", [128, T], BF16, isOutput=True)
        dbg_vb = nc.declare_dram_parameter("dbg_vb", [128, 16, 2, HS + 1], BF16, isOutput=True)
        dbg_osc = nc.declare_dram_parameter("dbg_osc", [128, T], BF16, isOutput=True)

    with tile.TileContext(nc) as tc:
        with (
            tc.tile_pool(name="const", bufs=1) as cpool,
            tc.tile_pool(name="qkv", bufs=2) as qkvpool,
            tc.tile_pool(name="xin", bufs=3) as xpool,
            tc.tile_pool(name="rope", bufs=3) as rpool,
            tc.tile_pool(name="pt", bufs=4) as ptpool,
            tc.tile_pool(name="osc", bufs=2) as opool,
            tc.tile_pool(name="aot", bufs=2) as aopool,
            tc.tile_pool(name="ysb", bufs=2) as ypool,
            tc.tile_pool(name="ps", bufs=2, space="PSUM") as psum,
        ):
            # ---- resident constants ----
            if SPLIT_QK:
                wqkh_sb = cpool.tile([128, 8, 256], FP8)
                nc.sync.dma_start(wqkh_sb[:], wqkh[:])
                wqkl_sb = cpool.tile([128, 8, 256], FP8)
                nc.sync.dma_start(wqkl_sb[:], wqkl[:])
            else:
                wqkb_sb = cpool.tile([128, 8, 256], BF16)
                nc.sync.dma_start(wqkb_sb[:], wqkb[:])
            wv_sb = cpool.tile([128, 8, 128], BF16)
            nc.sync.dma_start(wv_sb[:], wv[:])
            wo_sb = cpool.tile([128, C], BF16)
            nc.sync.dma_start(wo_sb[:], wo[:])
            bqk_sb = cpool.tile([128, 2], F32)
            nc.sync.dma_start(bqk_sb[:], bqk[:])
            cos_sb = cpool.tile([128, T], BF16)
            nc.sync.dma_start(cos_sb[:], cosT[:])
            sin_sb = cpool.tile([128, T], BF16)
            nc.sync.dma_start(sin_sb[:], sinP[:])
            pi_sb = cpool.tile([128, 128], BF16)
            nc.sync.dma_start(pi_sb[:], pim[:])
            ebias = cpool.tile([128, 1], F32)
            nc.gpsimd.memset(ebias[:], EXP_SHIFT)
            # causal-mask matmul constants: maskA.T @ maskB adds -3e4 to the
            # strict upper triangle (k > q) of a [128,128] S^T diagonal block
            maskA = cpool.tile([128, 128], BF16)
            nc.gpsimd.memset(maskA[:], -3.0e4)
            nc.gpsimd.affine_select(
                out=maskA[:], in_=maskA[:], compare_op=AX.is_ge,
                fill=0.0, base=0, pattern=[[1, 128]], channel_multiplier=-1)
            maskB = cpool.tile([128, 128], BF16)
            nc.gpsimd.memset(maskB[:], 0.0)
            nc.gpsimd.affine_select(
                out=maskB[:], in_=maskB[:], compare_op=AX.not_equal,
                fill=1.0, base=-1, pattern=[[-1, 128]], channel_multiplier=1)

            # manual ring of vb buffers, ones columns prefilled once
            vb_bufs = []
            for r in range(2):
                vbuf = cpool.tile([128, 16, 2, HS + 1], BF16, name=f"vb{r}")
                nc.gpsimd.memset(vbuf[:], 1.0)
                vb_bufs.append(vbuf)

            qkv_tiles = {}
            osc_tiles = {}

            def gen_proj(b):
                qT = qkvpool.tile([128, T], BF16, tag="qT", name=f"qT_{b}")
                kT = qkvpool.tile([128, T], BF16, tag="kT", name=f"kT_{b}")
                vb = vb_bufs[b % 2]
                qkv_tiles[b] = (qT, kT, vb)
                pending = [None]

                def flush():
                    if pending[0] is not None:
                        pending[0]()
                        pending[0] = None

                for ml in range(4):
                    tl = 512 * ml
                    if SPLIT_QK:
                        xt_h = xpool.tile([128, 8, 512], FP8, tag="xh", name=f"xh_{b}_{ml}")
                        nc.sync.dma_start(xt_h[:], xh[:, :, T * b + tl : T * b + tl + 512])
                        xt_l = xpool.tile([128, 8, 512], FP8, tag="xl", name=f"xl_{b}_{ml}")
                        nc.sync.dma_start(xt_l[:], xl[:, :, T * b + tl : T * b + tl + 512])
                    xt_b = xpool.tile([128, 8, 512], BF16, tag="xb", name=f"xb_{b}_{ml}")
                    nc.sync.dma_start(xt_b[:], xb[:, :, T * b + tl : T * b + tl + 512])

                    # --- q and k projections + rope ---
                    for which, dest in ((0, qT), (1, kT)):
                        ps = psum.tile([128, 512], F32, tag="sc", name=f"ps_{b}_{ml}_{which}")
                        wsl = slice(128 * which, 128 * which + 128)
                        if SPLIT_QK:
                            passes = ((xt_h, wqkh_sb), (xt_l, wqkh_sb), (xt_h, wqkl_sb))
                            n = 0
                            for xs, ws in passes:
                                for p in range(4):
                                    nc.tensor.matmul(
                                        ps[:],
                                        ws[:, 2 * p : 2 * p + 2, wsl],
                                        xs[:, 2 * p : 2 * p + 2, :],
                                        start=(n == 0), stop=(n == 11), perf_mode=DR)
                                    n += 1
                        else:
                            for g in range(8):
                                nc.tensor.matmul(
                                    ps[:], wqkb_sb[:, g, wsl], xt_b[:, g, :],
                                    start=(g == 0), stop=(g == 7))
                        bias = bqk_sb[:, which : which + 1]
                        xbar = rpool.tile([128, 512], BF16, tag="xbar",
                                          name=f"xbar_{b}_{ml}_{which}")
                        if SPLIT_QK:
                            nc.vector.tensor_scalar(
                                xbar[:], ps[:], 1.0 / WSCALE, bias,
                                op0=AX.mult, op1=AX.add)
                        else:
                            nc.vector.tensor_scalar_add(xbar[:], ps[:], bias)
                        u = rpool.tile([128, 512], BF16, tag="u", name=f"u_{b}_{ml}_{which}")
                        nc.gpsimd.tensor_tensor(u[:], xbar[:], sin_sb[:, tl : tl + 512],
                                                op=AX.mult)
                        t1 = rpool.tile([128, 512], BF16, tag="t1", name=f"t1_{b}_{ml}_{which}")
                        nc.gpsimd.tensor_tensor(t1[:], xbar[:], cos_sb[:, tl : tl + 512],
                                                op=AX.mult)
                        flush()

                        def make_tail(u=u, t1=t1, dest=dest, tl=tl, b=b, ml=ml, which=which):
                            def tail():
                                usw = psum.tile([128, 512], F32, tag="sc",
                                                name=f"usw_{b}_{ml}_{which}")
                                nc.tensor.matmul(usw[:], pi_sb[:], u[:],
                                                 start=True, stop=True)
                                nc.vector.tensor_tensor(dest[:, tl : tl + 512],
                                                        t1[:], usw[:], op=AX.add)
                            return tail

                        pending[0] = make_tail()
                        yield

                    # --- v projection, token-major (x stationary, bf16) ---
                    vps = psum.tile([128, 4, 2, HS], F32, tag="sc", name=f"vps_{b}_{ml}")
                    for tb in range(4):
                        for g in range(8):
                            nc.tensor.matmul(
                                vps[:, tb, :, :],
                                xt_b[:, g, 128 * tb : 128 * tb + 128],
                                wv_sb[:, g, :],
                                start=(tb == 0 and g == 0), stop=(g == 7),
                                skip_group_check=True)
                    nc.vector.tensor_copy(
                        vb[:, 4 * ml : 4 * ml + 4, :, 0:HS], vps[:])
                    yield
                flush()

            def gen_attn(b):
                qT, kT, vb = qkv_tiles[b]
                osc = opool.tile([128, T], BF16, tag="osc", name=f"osc_{b}")
                osc_tiles[b] = osc
                for j in range(2):
                    for h in range(HPC):
                        hr = slice(HS * h, HS * h + HS)
                        nkt = 8 * j + 8
                        ot0 = psum.tile([128, 512], F32, tag="ot", name=f"ot0_{b}_{j}_{h}")
                        ot1 = psum.tile([128, 512], F32, tag="ot", name=f"ot1_{b}_{j}_{h}")
                        ots = (ot0, ot1)
                        started = [False, False]
                        pt = None
                        qbase = 1024 * j
                        for kt in range(nkt):
                            if kt % 2 == 0:
                                pt = ptpool.tile([128, 2, 1024], BF16, tag="pt",
                                                 name=f"pt_{b}_{j}_{h}_{kt}")
                            o = max(0, (kt - 8 * j) * 128)
                            sp = psum.tile([128, 1024], F32, tag="sp",
                                           name=f"sp_{b}_{j}_{h}_{kt}")
                            if o < 512:
                                nc.tensor.matmul(
                                    sp[:, o:512],
                                    kT[hr, 128 * kt : 128 * kt + 128],
                                    qT[hr, qbase + o : qbase + 512],
                                    start=True, stop=True)
                            lo = max(o, 512)
                            nc.tensor.matmul(
                                sp[:, lo:1024],
                                kT[hr, 128 * kt : 128 * kt + 128],
                                qT[hr, qbase + lo : qbase + 1024],
                                start=True, stop=True)
                            if kt >= 8 * j:
                                nc.tensor.matmul(
                                    sp[:, o : o + 128], maskA[:], maskB[:],
                                    start=False, stop=True)
                            nc.scalar.activation(
                                pt[:, kt % 2, o:1024], sp[:, o:1024],
                                mybir.ActivationFunctionType.Exp,
                                bias=ebias[:], scale=1.0 / np.sqrt(HS))
                            if kt % 2 == 1:
                                # O accumulation for kt pair (kt-1, kt)
                                m = kt // 2
                                s_single = 2 * m - 8 * j
                                if 0 <= s_single < 8:
                                    ti, col = s_single // 4, 65 * (s_single % 4)
                                    nc.tensor.matmul(
                                        ots[ti][:, col : col + 65],
                                        pt[:, 0, 128 * s_single : 128 * s_single + 128],
                                        vb[:, 2 * m, h, :],
                                        start=(not started[ti]), stop=True,
                                        skip_group_check=True)
                                    started[ti] = True
                                for s in range(max(0, 2 * m + 1 - 8 * j), 8):
                                    ti, col = s // 4, 65 * (s % 4)
                                    last = (kt == min(nkt - 1, 8 * j + s))
                                    for i in range(2):
                                        nc.tensor.matmul(
                                            ots[ti][:, col : col + 65],
                                            pt[:, i, 128 * s : 128 * s + 128],
                                            vb[:, 2 * m + i, h, :],
                                            start=(not started[ti]),
                                            stop=(i == 1 and last),
                                            skip_group_check=True)
                                        started[ti] = True
                            yield
                        # drain: per-block scale by reciprocal of denominator
                        for s in range(8):
                            ti, col = s // 4, 65 * (s % 4)
                            tcol = 128 * (8 * j + s) + HS * h
                            rec = rpool.tile([128, 1], F32, tag="rec", bufs=4,
                                             name=f"rec_{b}_{j}_{h}_{s}")
                            nc.vector.reciprocal(rec[:], ots[ti][:, col + HS : col + HS + 1])
                            nc.vector.tensor_scalar_mul(
                                osc[:, tcol : tcol + HS],
                                ots[ti][:, col : col + HS], rec[:])
                        yield
                if debug and b == 0:
                    nc.sync.dma_start(dbg_qT[:], qT[:])
                    nc.sync.dma_start(dbg_kT[:], kT[:])
                    nc.sync.dma_start(dbg_vb[:], vb[:])
                    nc.sync.dma_start(dbg_osc[:], osc[:])

            def gen_out(b):
                osc = osc_tiles[b]
                ao = aopool.tile([128, T], BF16, tag="ao", name=f"ao_{b}")
                for g4 in range(4):
                    for t in range(4):
                        blk = 512 * g4 + 128 * t
                        nc.sync.dma_start_transpose(
                            ao[:, blk : blk + 128], osc[:, blk : blk + 128])
                    yield
                for ot_ in range(8):
                    ys = ypool.tile([128, T], BF16, tag="ys", name=f"ys_{b}_{ot_}")
                    for ml in range(4):
                        yp = psum.tile([128, 512], F32, tag="sc", name=f"yp_{b}_{ot_}_{ml}")
                        nc.tensor.matmul(
                            yp[:], wo_sb[:, 128 * ot_ : 128 * ot_ + 128],
                            ao[:, 512 * ml : 512 * ml + 512],
                            start=True, stop=True)
                        nc.vector.tensor_copy(ys[:, 512 * ml : 512 * ml + 512], yp[:])
                    nc.sync.dma_start(
                        yT[128 * ot_ : 128 * ot_ + 128, T * b : T * b + T], ys[:])
                    yield

            def pump(g):
                if g is None:
                    return None
                try:
                    next(g)
                    return g
                except StopIteration:
                    return None

            def drain(g):
                if g is not None:
                    for _ in g:
                        pass

            # ---- pipelined schedule ----
            proj_g = gen_proj(0)
            drain(proj_g)
            out_g = None
            for b in range(B):
                attn_g = gen_attn(b)
                proj_g = gen_proj(b + 1) if b + 1 < B else None
                aux = 0
                while attn_g is not None:
                    for _ in range(2):
                        attn_g = pump(attn_g)
                        if attn_g is None:
                            break
                    if aux % 2 == 0:
                        proj_g = pump(proj_g)
                    else:
                        out_g = pump(out_g)
                    aux += 1
                # attn(b) fully emitted; finish leftovers
                drain(out_g)
                out_g = gen_out(b)
                if b == B - 1:
                    drain(out_g)
                else:
                    drain(proj_g)
                if b - 1 in osc_tiles:
                    del osc_tiles[b - 1]
                if b in qkv_tiles:
                    del qkv_tiles[b]
    nc.compile()
    return nc


_NC_CACHE = None


def _get_nc():
    global _NC_CACHE
    if _NC_CACHE is None:
        _NC_CACHE = build_nc()
    return _NC_CACHE


def _bf(a):
    return np.ascontiguousarray(a.astype(ml_dtypes.bfloat16))


def _f8(a):
    return np.ascontiguousarray(a.astype(ml_dtypes.float8_e4m3))


def _grp(a):
    """(C, N) -> (128, 8, N) channel-grouped layout."""
    return a.reshape(8, 128, a.shape[1]).transpose(1, 0, 2)


def _prep_inputs(x, Wqkv, bqkv):
    """Host-side shard prep. Returns list of per-core input dicts."""
    xT = x.reshape(NT, C).T                      # (C, NT)
    xg = _grp(xT)                                # (128, 8, NT)
    xh_np = xg.astype(ml_dtypes.float8_e4m3)
    xl_np = (xg - xh_np.astype(np.float32)).astype(ml_dtypes.float8_e4m3)
    xb_np = _bf(xg)

    half = HS // 2
    thetas = 10000.0 ** (-np.arange(half, dtype=np.float64) / half)
    ang = np.arange(T, dtype=np.float64)[:, None] * thetas[None, :]   # (T, 32)
    sin = np.sin(ang).T.astype(np.float32)    # (32, T)
    cos = np.cos(ang).T.astype(np.float32)
    cosT = _bf(np.tile(cos, (4, 1)))
    sinP = _bf(np.concatenate([sin, -sin, sin, -sin], axis=0))

    pim = np.zeros((128, 128), dtype=np.float32)
    pim[np.arange(128), np.arange(128) ^ 32] = 1.0
    pim = _bf(pim)

    perm = np.concatenate([np.arange(0, HS, 2), np.arange(1, HS, 2)])  # de-interleave

    in_maps = []
    for c in range(NCORES):
        h0, h1 = 2 * c, 2 * c + 1
        wq = np.concatenate(
            [Wqkv[:, HS * h0 : HS * h0 + HS][:, perm],
             Wqkv[:, HS * h1 : HS * h1 + HS][:, perm]], axis=1)
        wk = np.concatenate(
            [Wqkv[:, C + HS * h0 : C + HS * h0 + HS][:, perm],
             Wqkv[:, C + HS * h1 : C + HS * h1 + HS][:, perm]], axis=1)
        wqk_c = _grp(np.concatenate([wq, wk], axis=1) * WSCALE)   # (128, 8, 256)
        wh = wqk_c.astype(ml_dtypes.float8_e4m3)
        wl = (wqk_c - wh.astype(np.float32)).astype(ml_dtypes.float8_e4m3)
        wv_c = _bf(_grp(Wqkv[:, 2 * C + HS * h0 : 2 * C + HS * h0 + 2 * HS]))
        bq = np.concatenate([bqkv[HS * h0 : HS * h0 + HS][perm],
                             bqkv[HS * h1 : HS * h1 + HS][perm]])
        bk = np.concatenate([bqkv[C + HS * h0 : C + HS * h0 + HS][perm],
                             bqkv[C + HS * h1 : C + HS * h1 + HS][perm]])
        bqk_c = np.ascontiguousarray(np.stack([bq, bk], axis=1).astype(np.float32))
        m = {
            "xb": xb_np,
            "wv": wv_c,
            "bqk": bqk_c,
            "cosT": cosT,
            "sinP": sinP,
            "pim": pim,
        }
        if SPLIT_QK:
            m.update({"xh": np.ascontiguousarray(xh_np),
                      "xl": np.ascontiguousarray(xl_np),
                      "wqkh": np.ascontiguousarray(wh),
                      "wqkl": np.ascontiguousarray(wl)})
        else:
            m["wqkb"] = _bf(wqk_c / WSCALE)
        in_maps.append(m)
    return in_maps


def kernel(x, Wqkv, bqkv, Wout, bout, num_heads):
    x = np.asarray(x, dtype=np.float32)
    Wqkv = np.asarray(Wqkv, dtype=np.float32)
    bqkv = np.asarray(bqkv, dtype=np.float32)
    Wout = np.asarray(Wout, dtype=np.float32)
    bout = np.asarray(bout, dtype=np.float32)

    nc = _get_nc()
    in_maps = _prep_inputs(x, Wqkv, bqkv)
    for c in range(NCORES):
        in_maps[c]["wo"] = _bf(Wout[128 * c : 128 * c + 128, :])

    res = run_bass_kernel_spmd(nc, in_maps, core_ids=list(range(NCORES)))

    acc = np.zeros((C, NT), dtype=np.float64)
    for c in range(NCORES):
        acc += res.results[c]["yT"].astype(np.float64)
    y = acc.T.astype(np.float32)                        # (NT, C)
    # biases: bout plus the folded V-bias contribution bv @ Wout
    bv = bqkv[2 * C : 3 * C]
    y += (bout + bv @ Wout)[None, :]
    return y.reshape(B, T, C)


if __name__ == "__main__":
    rng = np.random.default_rng(0)
    x = rng.standard_normal((B, T, C), dtype=np.float32)
    Wqkv = rng.standard_normal((C, 3 * C), dtype=np.float32) / 32
    bqkv = rng.standard_normal((3 * C,), dtype=np.float32) * 0.01
    Wout = rng.standard_normal((C, C), dtype=np.float32) / 32
    bout = rng.standard_normal((C,), dtype=np.float32) * 0.01
    y = kernel(x=x, Wqkv=Wqkv, bqkv=bqkv, Wout=Wout, bout=bout, num_heads=H)
    print("kernel output", y.shape, y.dtype, np.abs(y).mean())
